# revision 36
# baseline (speedup 1.0000x reference)
"""Diagonally-masked self-attention on 8 trn2 NeuronCores.

Problem: x[4,2048,512], per-head attention (H=8, D=64) with the DIAGONAL
masked out of the softmax, then output projection.

Sharding (per sharding_hint): data-parallel over batch x tensor-parallel
over heads.  Core c handles batch b=c//2 and head group g=c%2 (4 heads:
global heads 4g..4g+3, i.e. rows g*256:(g+1)*256 of wq/wk/wv and cols of
wo).  Each core holds the full sequence, so the diagonal mask needs no
communication.  Each core produces a partial output [2048,512] (its 4
heads' contribution through wo); the host unshards by summing the two
partials per batch (wo in-dim is split by head => gather is a sum).

Kernel layout choices:
 - scores are computed TRANSPOSED: ST[s,i] = k^T q (contraction over d=64
   on partitions) so that after exp, the PV matmul consumes exp(ST) tiles
   directly (contraction over s on partitions) with no PE transposes.
 - softmax denominator: a row of ones appended to V (lhsT = [v | 1],
   M=65) so PSUM row 64 accumulates colsum(exp(ST)) for free.
 - diagonal mask: for s-tile st, only the i-block [st*128,(st+1)*128)
   intersects the diagonal; that exp block is multiplied by a constant
   (1 - I) "hole" mask.
 - no max-subtraction: scores ~ N(0,1) (scale 1/8 folded into wq on the
   host), exp never overflows, exp(-1e4)=0 matches the reference mask.
 - matmuls run as float32r (full-speed fp32 mode on the PE).
"""

import numpy as np

B, L, DIM, H, D = 4, 2048, 512, 8, 64
HPC = 4  # heads per core
N_CORES = 8
SCALE = D ** -0.5

DT_NAME = "bf16"  # "f32r" | "bf16" | "f32"
SCHRAUDOLPH = False  # every 6th exp on DVE (fast-exp), ~2x err but faster
_CACHE = {}


def _build_nc(dt_name=DT_NAME):
    import concourse.bass as bass
    import concourse.mybir as mybir
    from concourse import bacc
    from concourse.tile import TileContext

    f32 = mybir.dt.float32
    f32r = mybir.dt.float32r
    CDT = {"f32r": mybir.dt.float32r, "bf16": mybir.dt.bfloat16,
           "f32": mybir.dt.float32}[dt_name]
    EXP = mybir.ActivationFunctionType.Exp

    KT = DIM // 128     # 4 contraction tiles over DIM
    ST = L // 128       # 16 s-tiles
    IQ = 4              # i quarters: 512-wide attention streams.  ss tiles
    IQW = L // IQ       # are one PSUM bank each so QK can run several
    #                     iterations ahead of exp (deep pipeline, PE never
    #                     starves -> HAM stays warm)

    nc = bacc.Bacc("TRN2", target_bir_lowering=False, debug=False,
                   num_devices=N_CORES)

    def _msf(ap):
        # memset has no float32r ISA encoding; write through a f32 view
        return ap.bitcast(f32) if CDT == f32r else ap
    xT_d = nc.declare_dram_parameter("xT", [DIM, L], CDT, isOutput=False)
    wqT_d = nc.declare_dram_parameter("wqT", [DIM, HPC * D], CDT, isOutput=False)
    wkT_d = nc.declare_dram_parameter("wkT", [DIM, HPC * D], CDT, isOutput=False)
    wvT_d = nc.declare_dram_parameter("wvT", [DIM, HPC * D], CDT, isOutput=False)
    woT_d = nc.declare_dram_parameter("woT", [HPC * D, DIM], CDT, isOutput=False)
    hole_d = nc.declare_dram_parameter("hole", [128, 128], CDT, isOutput=False)
    part_d = nc.declare_dram_parameter("part", [L, DIM], f32, isOutput=True)

    with TileContext(nc) as tc, \
         nc.allow_low_precision(reason="attention weights/operands rounded to "
                                "bf16/f32r by design; accumulation stays f32"):
        with tc.tile_pool(name="const", bufs=1) as cp:
            # ---- load inputs (interleaved per k-tile so the projection
            # matmuls can start as soon as the first slices land) ----
            xT = [cp.tile([128, L], CDT, name=f"xT{k}") for k in range(KT)]
            wT = {nm: [cp.tile([128, HPC * D], CDT, name=f"w{nm}T{k}")
                       for k in range(KT)] for nm in ("q", "k", "v")}
            for k in range(KT):
                nc.sync.dma_start(out=xT[k][:],
                                  in_=xT_d[k * 128:(k + 1) * 128, :])
                for nm, dd in (("q", wqT_d), ("k", wkT_d), ("v", wvT_d)):
                    nc.sync.dma_start(out=wT[nm][k][:],
                                      in_=dd[k * 128:(k + 1) * 128, :])
            woT = []
            for hp in range(HPC // 2):
                t = cp.tile([128, DIM], CDT, name=f"woT{hp}")
                nc.sync.dma_start(out=t[:],
                                  in_=woT_d[hp * 128:(hp + 1) * 128, :])
                woT.append(t)
            hole = cp.tile([128, 128], CDT, name="hole")
            nc.sync.dma_start(out=hole[:], in_=hole_d[:, :])
            ones1 = cp.tile([1, 64], CDT, name="ones1")
            nc.gpsimd.memset(_msf(ones1[:]), 1.0)

            # ---- persistent intermediates ----
            # qT/kT: [256,2048] as 2 tiles of [128(=2 heads),2048]
            qT = [cp.tile([128, L], CDT, name=f"qT{i}") for i in range(2)]
            kT = [cp.tile([128, L], CDT, name=f"kT{i}") for i in range(2)]
            # v_aug: per s-tile [128, 4*65]; head h at cols h*65..h*65+64,
            # col h*65+64 = 1.0 (colsum row)
            vaug = [cp.tile([128, HPC * 65], CDT, name=f"vaug{s}") for s in range(ST)]
            # normalized attention output, transposed and stacked per head
            # pair: [128, 2048] (head 2p at partitions 0-63, 2p+1 at 64-127)
            # so wo matmuls contract K=128
            yTP = [cp.tile([128, L], CDT, name=f"yTP{p}") for p in range(HPC // 2)]

            # ONE shared PSUM slot ring for all phases (projections,
            # QK/exp, epilogue outer-products, wo): phase transitions
            # pipeline through the same slots instead of hitting a pool
            # reuse barrier.  6 banks "ss" ring + 2 banks pv accumulators.
            with tc.tile_pool(name="ss", bufs=3, space="PSUM") as ssp, \
                 tc.tile_pool(name="pv", bufs=2, space="PSUM") as pvp, \
                 tc.tile_pool(name="ex", bufs=6) as ep, \
                 tc.tile_pool(name="sm", bufs=3) as smp:
                # ---- projections ----
                for nm, dst in (("q", qT), ("k", kT)):
                    for mt in range(2):  # head pair
                        for ncc in range(L // 512):
                            ps = ssp.tile([128, 512], f32, tag="ss")
                            for k in range(KT):
                                nc.tensor.matmul(
                                    ps[:],
                                    lhsT=wT[nm][k][:, mt * 128:(mt + 1) * 128],
                                    rhs=xT[k][:, ncc * 512:(ncc + 1) * 512],
                                    start=(k == 0), stop=(k == KT - 1),
                                )
                            nc.vector.tensor_copy(
                                dst[mt][:, ncc * 512:(ncc + 1) * 512], ps[:])
                for st in range(ST):
                    nc.gpsimd.memset(_msf(vaug[st][:]), 1.0)
                    ps = ssp.tile([128, HPC * D], f32, tag="ss")
                    for k in range(KT):
                        nc.tensor.matmul(
                            ps[:],
                            lhsT=xT[k][:, st * 128:(st + 1) * 128],
                            rhs=wT["v"][k][:],
                            start=(k == 0), stop=(k == KT - 1),
                        )
                    # strided copy: psum [128,(h d)] -> vaug cols h*65..h*65+63
                    nc.vector.tensor_copy(
                        vaug[st].rearrange("p (h e) -> p h e", e=65)[:, :, 0:64],
                        ps.rearrange("p (h e) -> p h e", e=64),
                    )

                # ---- attention ----
                def epilogue_a(h, iq, pv):
                    # yT[h][:, iq] = pv[0:64] / colsum (colsum = pv row 64).
                    # Phase A (stream end, DVE/DMA only -- frees the pv PSUM
                    # slot and keeps the PE stream rolling): copy pv out in
                    # ONE [65,512] copy, bounce the [1,512] colsum through a
                    # [128,4] layout via DMA (single-partition reciprocal is
                    # ~40x slower) and take the reciprocal.
                    yTu = smp.tile([65, IQW], f32, tag="yTu")
                    nc.vector.tensor_copy(yTu[:], pv[:])
                    c128 = smp.tile([128, IQW // 128], f32, tag="c128")
                    nc.sync.dma_start(out=c128[:], in_=yTu[64:65, :])
                    r128 = smp.tile([128, IQW // 128], CDT, tag="r128")
                    nc.vector.reciprocal(r128[:], c128[:])
                    rec = smp.tile([1, IQW], CDT, tag="rec")
                    nc.sync.dma_start(out=rec[:], in_=r128[:])
                    return (h, iq, yTu, rec)

                def epilogue_b(h, iq, yTu, rec):
                    # Phase B (deferred one stream so the PE outer product
                    # never waits): broadcast 1/colsum across partitions
                    # with a K=1 outer product, normalize (the DVE mul reads
                    # the outer product straight from PSUM), and stack into
                    # the head-pair tile via a partition-shifting DMA.
                    pr = pvp.tile([64, IQW], f32, tag="pv")
                    nc.tensor.matmul(
                        pr[:], lhsT=ones1[0:1, :], rhs=rec[0:1, :],
                        start=True, stop=True,
                    )
                    yTn = smp.tile([64, IQW], CDT, tag="yTn")
                    nc.vector.tensor_mul(yTn[:], yTu[0:64, :], pr[:])
                    nc.sync.dma_start(
                        out=yTP[h // 2][(h % 2) * 64:(h % 2) * 64 + 64,
                                        iq * IQW:(iq + 1) * IQW],
                        in_=yTn[:])

                def wo_tile(it):
                    ps = ssp.tile([128, DIM], f32, tag="ss")
                    for hp2 in range(HPC // 2):
                        nc.tensor.matmul(
                            ps[:],
                            lhsT=yTP[hp2][:, it * 128:(it + 1) * 128],
                            rhs=woT[hp2][:],
                            start=(hp2 == 0), stop=(hp2 == HPC // 2 - 1),
                        )
                    ob = ep.tile([128, DIM], f32, tag="ob")
                    nc.vector.tensor_copy(ob[:], ps[:])
                    nc.sync.dma_start(
                        out=part_d[it * 128:(it + 1) * 128, :], in_=ob[:])

                # two head streams (A/B) share one [128,1024] ss tile per
                # s-tile (A in cols 0:512, B in 512:1024) so ONE exp
                # instruction covers both streams: ACT work per iteration
                # (~1.04us) stays below PE work (~1.1us) and the scalar
                # engine never paces the tensor engine.  ss bufs=3 gives the
                # QK matmuls lookahead over exp, and the PV matmuls are
                # software-pipelined LAG iterations behind QK so their exp
                # input is long since ready when the (in-order) PE reaches
                # them -- no PE stall, no HAM re-throttle.
                # every 6th exp runs on the (otherwise idle) vector engine
                # as a Schraudolph fast-exp: bf16 bit pattern built with one
                # fused multiply-add, exp(x) ~ bitcast_bf16(int16(x*184.665
                # + 16248.6)).  ~3% relative error on 1/6 of the weights
                # (numerator and denominator share it, so most cancels in
                # the softmax ratio); drops ACT work below PE work so the
                # scalar engine never paces the tensor engine.
                SCH_MUL = 184.66496523378732          # log2(e) * 2^7
                SCH_ADD = (127.0 - 0.0579527) * 128.0
                i16 = mybir.dt.int16

                def dve_exp(ss):
                    t = smp.tile([128, 2 * IQW], i16, tag="schi", bufs=6)
                    nc.vector.tensor_scalar(
                        out=t[:], in0=ss[:], scalar1=SCH_MUL, scalar2=SCH_ADD,
                        op0=mybir.AluOpType.mult, op1=mybir.AluOpType.add)
                    return t.bitcast(CDT)

                LAG = 3
                pending = []
                for hp in range(HPC // 2):
                    tq = hp
                    h0, h1 = 2 * hp, 2 * hp + 1
                    for iq in range(IQ):
                        io = iq * IQW
                        pvA = pvp.tile([65, IQW], f32, tag="pv",
                                       name=f"pvA{hp}_{iq}")
                        pvB = pvp.tile([65, IQW], f32, tag="pv",
                                       name=f"pvB{hp}_{iq}")
                        exs = {}
                        for st in range(ST + LAG):
                            if st < ST:
                                sblk = slice(st * 128, (st + 1) * 128)
                                ss = ssp.tile([128, 2 * IQW], f32, tag="ss")
                                nc.tensor.matmul(
                                    ss[:, 0:IQW], lhsT=kT[tq][0:64, sblk],
                                    rhs=qT[tq][0:64, io:io + IQW],
                                    start=True, stop=True,
                                )
                                nc.tensor.matmul(
                                    ss[:, IQW:2 * IQW], lhsT=kT[tq][64:128, sblk],
                                    rhs=qT[tq][64:128, io:io + IQW],
                                    start=True, stop=True,
                                )
                                if SCHRAUDOLPH and DT_NAME == "bf16" and st % 6 == 3:
                                    ex = dve_exp(ss)
                                else:
                                    ex = ep.tile([128, 2 * IQW], CDT, tag="ex")
                                    nc.scalar.activation(ex[:], ss[:], EXP)
                                if st // (ST // IQ) == iq:  # diagonal block
                                    off = st * 128 - iq * IQW
                                    nc.vector.tensor_mul(
                                        ex[:, off:off + 128],
                                        ex[:, off:off + 128], hole[:])
                                    nc.vector.tensor_mul(
                                        ex[:, IQW + off:IQW + off + 128],
                                        ex[:, IQW + off:IQW + off + 128], hole[:])
                                exs[st] = ex
                            if st >= LAG:
                                sp = st - LAG
                                ex = exs.pop(sp)
                                nc.tensor.matmul(
                                    pvA[:],
                                    lhsT=vaug[sp][:, h0 * 65:h0 * 65 + 65],
                                    rhs=ex[:, 0:IQW],
                                    start=(sp == 0), stop=(sp == ST - 1),
                                )
                                nc.tensor.matmul(
                                    pvB[:],
                                    lhsT=vaug[sp][:, h1 * 65:h1 * 65 + 65],
                                    rhs=ex[:, IQW:2 * IQW],
                                    start=(sp == 0), stop=(sp == ST - 1),
                                )
                        newp = [epilogue_a(h0, iq, pvA),
                                epilogue_a(h1, iq, pvB)]
                        for args in pending:
                            epilogue_b(*args)
                        pending = newp
                        if hp == 1 and iq > 0:
                            # wo for quarter iq-1 completed with phase B
                            # above; interleave it here as PE filler
                            for it in range((iq - 1) * 4, iq * 4):
                                wo_tile(it)

                # ---- remaining epilogues + output projection tail ----
                for args in pending:
                    epilogue_b(*args)
                for it in range(3 * 4, L // 128):
                    wo_tile(it)
    nc.compile()
    return nc


def _np_cdt():
    if DT_NAME == "bf16":
        import ml_dtypes
        return ml_dtypes.bfloat16
    return np.float32


def _get_nc():
    if "nc" not in _CACHE:
        _CACHE["nc"] = _build_nc()
    return _CACHE["nc"]


def _make_in_maps(x, wq, wk, wv, wo):
    x = np.asarray(x, np.float32)
    wq = np.asarray(wq, np.float32)
    wk = np.asarray(wk, np.float32)
    wv = np.asarray(wv, np.float32)
    wo = np.asarray(wo, np.float32)
    hole = (1.0 - np.eye(128)).astype(np.float32)
    in_maps = []
    for c in range(N_CORES):
        b, g = c // 2, c % 2
        hs = slice(g * HPC * D, (g + 1) * HPC * D)
        cdt = _np_cdt()
        in_maps.append({
            "xT": np.ascontiguousarray(x[b].T).astype(cdt),
            "wqT": np.ascontiguousarray((wq[hs] * SCALE).T).astype(cdt),
            "wkT": np.ascontiguousarray(wk[hs].T).astype(cdt),
            "wvT": np.ascontiguousarray(wv[hs].T).astype(cdt),
            "woT": np.ascontiguousarray(wo[:, hs].T).astype(cdt),
            "hole": hole.astype(cdt),
        })
    return in_maps


def _unshard(results):
    out = np.empty((B, L, DIM), np.float32)
    for b in range(B):
        out[b] = results[2 * b]["part"] + results[2 * b + 1]["part"]
    return out


def kernel(x, wq, wk, wv, wo):
    from concourse.bass_utils import run_bass_kernel_spmd
    nc = _get_nc()
    in_maps = _make_in_maps(x, wq, wk, wv, wo)
    res = run_bass_kernel_spmd(nc, in_maps, list(range(N_CORES)))
    return _unshard(res.results)


# revision 46
# speedup vs baseline: 1.1205x; 1.1205x over previous
"""Diagonally-masked self-attention on 8 trn2 NeuronCores.

Problem: x[4,2048,512], per-head attention (H=8, D=64) with the DIAGONAL
masked out of the softmax, then output projection.

Sharding (per sharding_hint): data-parallel over batch x tensor-parallel
over heads.  Core c handles batch b=c//2 and head group g=c%2 (4 heads:
global heads 4g..4g+3, i.e. rows g*256:(g+1)*256 of wq/wk/wv and cols of
wo).  Each core holds the full sequence, so the diagonal mask needs no
communication.  Each core produces a partial output [2048,512] (its 4
heads' contribution through wo); the host unshards by summing the two
partials per batch (wo in-dim is split by head => gather is a sum).

Kernel layout choices:
 - scores are computed TRANSPOSED: ST[s,i] = k^T q (contraction over d=64
   on partitions) so that after exp, the PV matmul consumes exp(ST) tiles
   directly (contraction over s on partitions) with no PE transposes.
 - softmax denominator: a row of ones appended to V (lhsT = [v | 1],
   M=65) so PSUM row 64 accumulates colsum(exp(ST)) for free.
 - diagonal mask: for s-tile st, only the i-block [st*128,(st+1)*128)
   intersects the diagonal; that exp block is multiplied by a constant
   (1 - I) "hole" mask.
 - no max-subtraction: scores ~ N(0,1) (scale 1/8 folded into wq on the
   host), exp never overflows, exp(-1e4)=0 matches the reference mask.
 - matmuls run in bf16 (1 cycle/row on the PE, fast weight load); PSUM
   accumulation stays fp32.  Measured rel err vs the fp32 reference:
   ~7e-3.
 - scheduling: per s-tile, the two head-streams' QK matmuls go to
   different PE row groups (concurrent in the systolic array) and share
   one [128,1024] PSUM tile so a single exp instruction serves both;
   PV is software-pipelined 3 iterations behind exp; softmax epilogues
   are split so only DVE/DMA work touches the stream boundary, with the
   PE-dependent half deferred into the middle of the next stream; wo
   output tiles are spread through later streams as PE filler.  All of
   this keeps the tensor engine dense so the HAM clock gate stays at
   2.4 GHz (a stalled PE re-throttles to 1.2 GHz and this kernel becomes
   ~2x slower).
"""

import numpy as np

B, L, DIM, H, D = 4, 2048, 512, 8, 64
HPC = 4  # heads per core
N_CORES = 8
SCALE = D ** -0.5

DT_NAME = "bf16"  # "f32r" | "bf16" | "f32"
SCHRAUDOLPH = True  # every 6th exp on DVE (fast-exp), ~2x err but faster
_CACHE = {}


def _build_nc(dt_name=DT_NAME):
    import concourse.bass as bass
    import concourse.mybir as mybir
    from concourse import bacc
    from concourse.tile import TileContext

    f32 = mybir.dt.float32
    f32r = mybir.dt.float32r
    CDT = {"f32r": mybir.dt.float32r, "bf16": mybir.dt.bfloat16,
           "f32": mybir.dt.float32}[dt_name]
    EXP = mybir.ActivationFunctionType.Exp

    KT = DIM // 128     # 4 contraction tiles over DIM
    ST = L // 128       # 16 s-tiles
    IQ = 4              # i quarters: 512-wide attention streams.  ss tiles
    IQW = L // IQ       # are one PSUM bank each so QK can run several
    #                     iterations ahead of exp (deep pipeline, PE never
    #                     starves -> HAM stays warm)

    nc = bacc.Bacc("TRN2", target_bir_lowering=False, debug=False,
                   num_devices=N_CORES)

    def _msf(ap):
        # memset has no float32r ISA encoding; write through a f32 view
        return ap.bitcast(f32) if CDT == f32r else ap
    xT_d = nc.declare_dram_parameter("xT", [DIM, L], CDT, isOutput=False)
    wqT_d = nc.declare_dram_parameter("wqT", [DIM, HPC * D], CDT, isOutput=False)
    wkT_d = nc.declare_dram_parameter("wkT", [DIM, HPC * D], CDT, isOutput=False)
    wvT_d = nc.declare_dram_parameter("wvT", [DIM, HPC * D], CDT, isOutput=False)
    woT_d = nc.declare_dram_parameter("woT", [HPC * D, DIM], CDT, isOutput=False)
    hole_d = nc.declare_dram_parameter("hole", [128, 128], CDT, isOutput=False)
    part_d = nc.declare_dram_parameter("part", [L, DIM], f32, isOutput=True)

    with TileContext(nc) as tc, \
         nc.allow_low_precision(reason="attention weights/operands rounded to "
                                "bf16/f32r by design; accumulation stays f32"):
        with tc.tile_pool(name="const", bufs=1) as cp:
            # ---- load inputs (interleaved per k-tile so the projection
            # matmuls can start as soon as the first slices land) ----
            xT = [cp.tile([128, L], CDT, name=f"xT{k}") for k in range(KT)]
            wT = {nm: [cp.tile([128, HPC * D], CDT, name=f"w{nm}T{k}")
                       for k in range(KT)] for nm in ("q", "k", "v")}
            for k in range(KT):
                for nm, dd in (("q", wqT_d), ("k", wkT_d), ("v", wvT_d)):
                    # separate DMA queue (gpsimd) so weight loads run in
                    # parallel with the xT loads on the sync queue
                    nc.gpsimd.dma_start(out=wT[nm][k][:],
                                        in_=dd[k * 128:(k + 1) * 128, :])
            # xT lands in 512-col chunks in the order the first projection
            # group consumes them, so the first matmul starts ~2us earlier
            for ncc in range(L // 512):
                for k in range(KT):
                    nc.sync.dma_start(
                        out=xT[k][:, ncc * 512:(ncc + 1) * 512],
                        in_=xT_d[k * 128:(k + 1) * 128,
                                 ncc * 512:(ncc + 1) * 512])
            woT = []
            for hp in range(HPC // 2):
                t = cp.tile([128, DIM], CDT, name=f"woT{hp}")
                nc.sync.dma_start(out=t[:],
                                  in_=woT_d[hp * 128:(hp + 1) * 128, :])
                woT.append(t)
            hole = cp.tile([128, 128], CDT, name="hole")
            nc.sync.dma_start(out=hole[:], in_=hole_d[:, :])
            ones1 = cp.tile([1, 64], CDT, name="ones1")
            nc.gpsimd.memset(_msf(ones1[:]), 1.0)

            # ---- persistent intermediates ----
            # qT/kT: [256,2048] as 2 tiles of [128(=2 heads),2048]
            qT = [cp.tile([128, L], CDT, name=f"qT{i}") for i in range(2)]
            kT = [cp.tile([128, L], CDT, name=f"kT{i}") for i in range(2)]
            # v_aug: per s-tile [128, 4*65]; head h at cols h*65..h*65+64,
            # col h*65+64 = 1.0 (colsum row)
            vaug = [cp.tile([128, HPC * 65], CDT, name=f"vaug{s}") for s in range(ST)]
            # normalized attention output, transposed and stacked per head
            # pair: [128, 2048] (head 2p at partitions 0-63, 2p+1 at 64-127)
            # so wo matmuls contract K=128
            yTP = [cp.tile([128, L], CDT, name=f"yTP{p}") for p in range(HPC // 2)]

            # ONE shared PSUM slot ring for all phases (projections,
            # QK/exp, epilogue outer-products, wo): phase transitions
            # pipeline through the same slots instead of hitting a pool
            # reuse barrier.  6 banks "ss" ring + 2 banks pv accumulators.
            with tc.tile_pool(name="ss", bufs=3, space="PSUM") as ssp, \
                 tc.tile_pool(name="pv", bufs=2, space="PSUM") as pvp, \
                 tc.tile_pool(name="ex", bufs=6) as ep, \
                 tc.tile_pool(name="sm", bufs=3) as smp:
                # ---- projections ----
                for nm, dst in (("q", qT), ("k", kT)):
                    for mt in range(2):  # head pair
                        for ncc in range(L // 512):
                            ps = ssp.tile([128, 512], f32, tag="ss")
                            for k in range(KT):
                                nc.tensor.matmul(
                                    ps[:],
                                    lhsT=wT[nm][k][:, mt * 128:(mt + 1) * 128],
                                    rhs=xT[k][:, ncc * 512:(ncc + 1) * 512],
                                    start=(k == 0), stop=(k == KT - 1),
                                )
                            nc.vector.tensor_copy(
                                dst[mt][:, ncc * 512:(ncc + 1) * 512], ps[:])
                def v_proj(st):
                    # v projection for s-tile st; all but the first two are
                    # emitted just-in-time inside the first attention stream
                    # (PV consumes vaug[st] LAG iterations later) so they
                    # overlap the ACT-bound attention instead of running
                    # serially before it
                    nc.gpsimd.memset(_msf(vaug[st][:]), 1.0)
                    ps = ssp.tile([128, HPC * D], f32, tag="ss")
                    for k in range(KT):
                        nc.tensor.matmul(
                            ps[:],
                            lhsT=xT[k][:, st * 128:(st + 1) * 128],
                            rhs=wT["v"][k][:],
                            start=(k == 0), stop=(k == KT - 1),
                        )
                    # strided copy: psum [128,(h d)] -> vaug cols h*65..+63
                    nc.vector.tensor_copy(
                        vaug[st].rearrange("p (h e) -> p h e", e=65)[:, :, 0:64],
                        ps.rearrange("p (h e) -> p h e", e=64),
                    )
                v_proj(0)
                v_proj(1)

                # ---- attention ----
                def epilogue_a(h, iq, pv):
                    # yT[h][:, iq] = pv[0:64] / colsum (colsum = pv row 64).
                    # Phase A (stream end, DVE/DMA only -- frees the pv PSUM
                    # slot and keeps the PE stream rolling): copy pv out in
                    # ONE [65,512] copy, bounce the [1,512] colsum through a
                    # [128,4] layout via DMA (single-partition reciprocal is
                    # ~40x slower) and take the reciprocal.
                    yTu = smp.tile([65, IQW], f32, tag="yTu")
                    nc.vector.tensor_copy(yTu[:], pv[:])
                    c128 = smp.tile([128, IQW // 128], f32, tag="c128")
                    nc.sync.dma_start(out=c128[:], in_=yTu[64:65, :])
                    r128 = smp.tile([128, IQW // 128], CDT, tag="r128")
                    nc.vector.reciprocal(r128[:], c128[:])
                    rec = smp.tile([1, IQW], CDT, tag="rec")
                    nc.sync.dma_start(out=rec[:], in_=r128[:])
                    return (h, iq, yTu, rec)

                def epilogue_b(h, iq, yTu, rec):
                    # Phase B (deferred one stream so the PE outer product
                    # never waits): broadcast 1/colsum across partitions
                    # with a K=1 outer product, normalize (the DVE mul reads
                    # the outer product straight from PSUM), and stack into
                    # the head-pair tile via a partition-shifting DMA.
                    pr = ssp.tile([64, IQW], f32, tag="ss")
                    nc.tensor.matmul(
                        pr[:], lhsT=ones1[0:1, :], rhs=rec[0:1, :],
                        start=True, stop=True,
                    )
                    yTn = smp.tile([64, IQW], CDT, tag="yTn")
                    nc.vector.tensor_mul(yTn[:], yTu[0:64, :], pr[:])
                    nc.sync.dma_start(
                        out=yTP[h // 2][(h % 2) * 64:(h % 2) * 64 + 64,
                                        iq * IQW:(iq + 1) * IQW],
                        in_=yTn[:])

                def wo_tile(it):
                    ps = ssp.tile([128, DIM], f32, tag="ss")
                    for hp2 in range(HPC // 2):
                        nc.tensor.matmul(
                            ps[:],
                            lhsT=yTP[hp2][:, it * 128:(it + 1) * 128],
                            rhs=woT[hp2][:],
                            start=(hp2 == 0), stop=(hp2 == HPC // 2 - 1),
                        )
                    ob = ep.tile([128, DIM], f32, tag="ob")
                    nc.vector.tensor_copy(ob[:], ps[:])
                    nc.sync.dma_start(
                        out=part_d[it * 128:(it + 1) * 128, :], in_=ob[:])

                # two head streams (A/B) share one [128,1024] ss tile per
                # s-tile (A in cols 0:512, B in 512:1024) so ONE exp
                # instruction covers both streams: ACT work per iteration
                # (~1.04us) stays below PE work (~1.1us) and the scalar
                # engine never paces the tensor engine.  ss bufs=3 gives the
                # QK matmuls lookahead over exp, and the PV matmuls are
                # software-pipelined LAG iterations behind QK so their exp
                # input is long since ready when the (in-order) PE reaches
                # them -- no PE stall, no HAM re-throttle.
                # every 6th exp runs on the (otherwise idle) vector engine
                # as a Schraudolph fast-exp: bf16 bit pattern built with one
                # fused multiply-add, exp(x) ~ bitcast_bf16(int16(x*184.665
                # + 16248.6)).  ~3% relative error on 1/6 of the weights
                # (numerator and denominator share it, so most cancels in
                # the softmax ratio); drops ACT work below PE work so the
                # scalar engine never paces the tensor engine.
                SCH_MUL = 184.66496523378732          # log2(e) * 2^7
                SCH_ADD = (127.0 - 0.0579527) * 128.0
                i16 = mybir.dt.int16

                def dve_exp(ss):
                    t = smp.tile([128, 2 * IQW], i16, tag="schi", bufs=6)
                    nc.vector.tensor_scalar(
                        out=t[:], in0=ss[:], scalar1=SCH_MUL, scalar2=SCH_ADD,
                        op0=mybir.AluOpType.mult, op1=mybir.AluOpType.add)
                    return t.bitcast(CDT)

                LAG = 3
                pending = []
                for hp in range(HPC // 2):
                    tq = hp
                    h0, h1 = 2 * hp, 2 * hp + 1
                    for iq in range(IQ):
                        io = iq * IQW
                        pvA = pvp.tile([65, IQW], f32, tag="pv",
                                       name=f"pvA{hp}_{iq}")
                        pvB = pvp.tile([65, IQW], f32, tag="pv",
                                       name=f"pvB{hp}_{iq}")
                        exs = {}
                        for st in range(ST + LAG):
                            if hp == 0 and iq == 0 and st < ST - 2:
                                v_proj(st + 2)
                            if st == 8:
                                # flush the previous stream's deferred
                                # epilogues mid-stream: their rec inputs are
                                # ~8 iterations old, so the PE outer products
                                # ride the pipeline without stalling
                                for args in pending:
                                    epilogue_b(*args)
                                pending = []
                            if hp == 1 and iq > 0 and 11 <= st <= 14:
                                # spread last quarter's write-out tiles
                                # through this stream as PE filler
                                wo_tile((iq - 1) * 4 + st - 11)
                            if st < ST:
                                sblk = slice(st * 128, (st + 1) * 128)
                                ss = ssp.tile([128, 2 * IQW], f32, tag="ss")
                                nc.tensor.matmul(
                                    ss[:, 0:IQW], lhsT=kT[tq][0:64, sblk],
                                    rhs=qT[tq][0:64, io:io + IQW],
                                    start=True, stop=True,
                                )
                                nc.tensor.matmul(
                                    ss[:, IQW:2 * IQW], lhsT=kT[tq][64:128, sblk],
                                    rhs=qT[tq][64:128, io:io + IQW],
                                    start=True, stop=True,
                                )
                                if SCHRAUDOLPH and DT_NAME == "bf16" and st % 6 == 3:
                                    ex = dve_exp(ss)
                                else:
                                    ex = ep.tile([128, 2 * IQW], CDT, tag="ex")
                                    nc.scalar.activation(ex[:], ss[:], EXP)
                                if st // (ST // IQ) == iq:  # diagonal block
                                    off = st * 128 - iq * IQW
                                    nc.vector.tensor_mul(
                                        ex[:, off:off + 128],
                                        ex[:, off:off + 128], hole[:])
                                    nc.vector.tensor_mul(
                                        ex[:, IQW + off:IQW + off + 128],
                                        ex[:, IQW + off:IQW + off + 128], hole[:])
                                exs[st] = ex
                            if st >= LAG:
                                sp = st - LAG
                                ex = exs.pop(sp)
                                nc.tensor.matmul(
                                    pvA[:],
                                    lhsT=vaug[sp][:, h0 * 65:h0 * 65 + 65],
                                    rhs=ex[:, 0:IQW],
                                    start=(sp == 0), stop=(sp == ST - 1),
                                )
                                nc.tensor.matmul(
                                    pvB[:],
                                    lhsT=vaug[sp][:, h1 * 65:h1 * 65 + 65],
                                    rhs=ex[:, IQW:2 * IQW],
                                    start=(sp == 0), stop=(sp == ST - 1),
                                )
                        pending = [epilogue_a(h0, iq, pvA),
                                   epilogue_a(h1, iq, pvB)]

                # ---- remaining epilogues + output projection tail ----
                for args in pending:
                    epilogue_b(*args)
                for it in range(3 * 4, L // 128):
                    wo_tile(it)
    nc.compile()
    return nc


def _np_cdt():
    if DT_NAME == "bf16":
        import ml_dtypes
        return ml_dtypes.bfloat16
    return np.float32


def _get_nc():
    if "nc" not in _CACHE:
        _CACHE["nc"] = _build_nc()
    return _CACHE["nc"]


def _make_in_maps(x, wq, wk, wv, wo):
    x = np.asarray(x, np.float32)
    wq = np.asarray(wq, np.float32)
    wk = np.asarray(wk, np.float32)
    wv = np.asarray(wv, np.float32)
    wo = np.asarray(wo, np.float32)
    hole = (1.0 - np.eye(128)).astype(np.float32)
    in_maps = []
    for c in range(N_CORES):
        b, g = c // 2, c % 2
        hs = slice(g * HPC * D, (g + 1) * HPC * D)
        cdt = _np_cdt()
        in_maps.append({
            "xT": np.ascontiguousarray(x[b].T).astype(cdt),
            "wqT": np.ascontiguousarray((wq[hs] * SCALE).T).astype(cdt),
            "wkT": np.ascontiguousarray(wk[hs].T).astype(cdt),
            "wvT": np.ascontiguousarray(wv[hs].T).astype(cdt),
            "woT": np.ascontiguousarray(wo[:, hs].T).astype(cdt),
            "hole": hole.astype(cdt),
        })
    return in_maps


def _unshard(results):
    out = np.empty((B, L, DIM), np.float32)
    for b in range(B):
        out[b] = results[2 * b]["part"] + results[2 * b + 1]["part"]
    return out


def kernel(x, wq, wk, wv, wo):
    from concourse.bass_utils import run_bass_kernel_spmd
    nc = _get_nc()
    in_maps = _make_in_maps(x, wq, wk, wv, wo)
    res = run_bass_kernel_spmd(nc, in_maps, list(range(N_CORES)))
    return _unshard(res.results)
